# revision 5
# baseline (speedup 1.0000x reference)
"""Trainium2 Bass kernel for nn_LookupFFN (vq_codebook).

reference:  proj = x @ R.T ; idx = argmax(proj, 1) ; out = L[idx]
  x: [16384, 1024] f32, R: [1024, 1024] f32, L: [1024, 1024] f32

Strategy (data-parallel over 8 NeuronCores, 2048 rows of x per core):
  - The argmax needs full-fp32-class precision (real top-2 margins go down
    to 6.8e-4 while proj ~ +-130): a plain bf16 or fp32r (tf32-class)
    matmul flips rows.  fp32 matmul runs at 1/4 PE rate, so instead use a
    3-term bf16 split computed on host:
        x = xh + xl, R = Rh + Rl  (bf16 splits)
        x @ R.T ~= xh@Rh.T + xh@Rl.T + xl@Rh.T   (error ~1e-4, fp32-class)
    All three terms accumulate into the same PSUM tile at full bf16 PE
    rate (3 cycles/row total vs 4 for fp32).
  - Row-max + argmax via VectorE max/max_index straight from PSUM.
  - out rows fetched exactly (fp32) with a GPSIMD indirect DMA gather of
    L rows by the computed indices.
Perf notes:
  - R splits are loaded as 16 separate per-k-tile chunk DMAs (not 2 big
    4MB DMAs) so the first matmuls start ~1.5us in instead of ~18us.
  - Term/k loop is outer, bucket-half inner: each stationary lhsT tile is
    loaded once and streams both 512-wide halves.
"""
import sys

if "/opt/trn_rl_repo" not in sys.path:
    sys.path.insert(0, "/opt/trn_rl_repo")

import ml_dtypes
import numpy as np

import concourse.bass as bass
import concourse.tile as tile
from concourse import bacc, mybir
from concourse.bass import IndirectOffsetOnAxis
from concourse.bass_utils import run_bass_kernel_spmd

F32 = mybir.dt.float32
BF16 = mybir.dt.bfloat16
U32 = mybir.dt.uint32

N = 16384
D = 1024
NB = 1024  # buckets
DOUT = 1024
NCORES = 8
NSHARD = N // NCORES  # 2048 rows per core
KT = D // 128  # 8 k-tiles
NTILES = NSHARD // 128  # 16 n-tiles per core

_CACHED = {}


def build_nc(n_bufs: int = 4):
    nc = bacc.Bacc("TRN2", target_bir_lowering=False, debug=False)
    xh = nc.declare_dram_parameter("xh", [D, NSHARD], BF16, isOutput=False)
    xl = nc.declare_dram_parameter("xl", [D, NSHARD], BF16, isOutput=False)
    rh = nc.declare_dram_parameter("rh", [D, NB], BF16, isOutput=False)
    rl = nc.declare_dram_parameter("rl", [D, NB], BF16, isOutput=False)
    L = nc.declare_dram_parameter("L", [NB, DOUT], F32, isOutput=False)
    out = nc.declare_dram_parameter("out", [NSHARD, DOUT], F32, isOutput=True)

    with tile.TileContext(nc) as tc:
        with (
            tc.tile_pool(name="rpool", bufs=1) as rpool,
            tc.tile_pool(name="xpool", bufs=n_bufs) as xpool,
            tc.tile_pool(name="gpool", bufs=n_bufs) as gpool,
            tc.tile_pool(name="ipool", bufs=n_bufs) as ipool,
            tc.tile_pool(name="ps", bufs=4, space="PSUM") as ps,
        ):
            # R splits resident in SBUF, one tile per k-chunk so the first
            # matmuls only wait on their own chunk's DMA. Issued on the
            # scalar engine's HWDGE queue so they don't serialize ahead of
            # the x-tile loads on the sync queue (~0.7us issue cost each).
            rh_sb = []
            rl_sb = []
            for k2 in range(KT // 2):
                t_ = rpool.tile([128, 2, NB], BF16, tag=f"rh{k2}")
                nc.scalar.dma_start(
                    out=t_[:],
                    in_=rh[k2 * 256 : (k2 + 1) * 256, :].rearrange(
                        "(k p) b -> p k b", k=2
                    ),
                )
                rh_sb.extend([t_[:, 0, :], t_[:, 1, :]])
            for k2 in range(KT // 2):
                t_ = rpool.tile([128, 2, NB], BF16, tag=f"rl{k2}")
                nc.scalar.dma_start(
                    out=t_[:],
                    in_=rl[k2 * 256 : (k2 + 1) * 256, :].rearrange(
                        "(k p) b -> p k b", k=2
                    ),
                )
                rl_sb.extend([t_[:, 0, :], t_[:, 1, :]])

            for t in range(NTILES):
                c0 = t * 128
                xh_sb = xpool.tile([128, KT, 128], BF16, tag="xh")
                xl_sb = xpool.tile([128, KT, 128], BF16, tag="xl")
                nc.sync.dma_start(
                    out=xh_sb[:],
                    in_=xh[:, c0 : c0 + 128].rearrange("(k p) j -> p k j", k=KT),
                )
                nc.sync.dma_start(
                    out=xl_sb[:],
                    in_=xl[:, c0 : c0 + 128].rearrange("(k p) j -> p k j", k=KT),
                )

                proj = ps.tile([128, NB], F32, tag="proj")
                # rh-consuming terms first: rl chunks arrive last on the
                # scalar DMA queue, so don't need them until 2/3 through.
                steps = (
                    [(xh_sb, rh_sb, k) for k in range(KT)]
                    + [(xl_sb, rh_sb, k) for k in range(KT)]
                    + [(xh_sb, rl_sb, k) for k in range(KT)]
                )
                n_steps = len(steps)
                for i, (xs, rs, k) in enumerate(steps):
                    for bh in range(2):
                        bs = bh * 512
                        nc.tensor.matmul(
                            proj[:, bs : bs + 512],
                            lhsT=xs[:, k, :],
                            rhs=rs[k][:, bs : bs + 512],
                            start=(i == 0),
                            stop=(i == n_steps - 1),
                        )

                max8 = ipool.tile([128, 8], F32, tag="max8")
                idx8 = ipool.tile([128, 8], U32, tag="idx8")
                nc.vector.max(max8[:], proj[:])
                nc.vector.max_index(idx8[:], max8[:], proj[:])

                g_sb = gpool.tile([128, DOUT], F32, tag="g")
                nc.gpsimd.indirect_dma_start(
                    out=g_sb[:],
                    out_offset=None,
                    in_=L[:],
                    in_offset=IndirectOffsetOnAxis(ap=idx8[:, 0:1], axis=0),
                )
                nc.sync.dma_start(out=out[c0 : c0 + 128, :], in_=g_sb[:])
    nc.compile()
    return nc


def _get_nc():
    if "nc" not in _CACHED:
        _CACHED["nc"] = build_nc()
    return _CACHED["nc"]


def _prep_inputs(x, R, L):
    """Host-side split + transpose. Returns per-core input maps."""
    x = np.ascontiguousarray(x, dtype=np.float32)
    R = np.ascontiguousarray(R, dtype=np.float32)
    L = np.ascontiguousarray(L, dtype=np.float32)

    xh = x.astype(ml_dtypes.bfloat16)
    xl = (x - xh.astype(np.float32)).astype(ml_dtypes.bfloat16)
    Rh = R.astype(ml_dtypes.bfloat16)
    Rl = (R - Rh.astype(np.float32)).astype(ml_dtypes.bfloat16)

    xhT = np.ascontiguousarray(xh.T)  # [D, N]
    xlT = np.ascontiguousarray(xl.T)
    rhT = np.ascontiguousarray(Rh.T)  # [D, NB]
    rlT = np.ascontiguousarray(Rl.T)

    in_maps = []
    for c in range(NCORES):
        s = slice(c * NSHARD, (c + 1) * NSHARD)
        in_maps.append(
            {
                "xh": np.ascontiguousarray(xhT[:, s]),
                "xl": np.ascontiguousarray(xlT[:, s]),
                "rh": rhT,
                "rl": rlT,
                "L": L,
            }
        )
    return in_maps


def run(x, R, L, trace=False, **kw):
    nc = _get_nc()
    in_maps = _prep_inputs(x, R, L)
    res = run_bass_kernel_spmd(
        nc, in_maps, core_ids=list(range(NCORES)), trace=trace, **kw
    )
    out = np.concatenate([res.results[c]["out"] for c in range(NCORES)], axis=0)
    return out, res


def kernel(x, R, L):
    out, _ = run(x, R, L, trace=False)
    return out


if __name__ == "__main__":
    rng = np.random.default_rng(0)
    x = rng.standard_normal((N, D), dtype=np.float32)
    R = rng.standard_normal((NB, D), dtype=np.float32)
    L = rng.standard_normal((NB, DOUT), dtype=np.float32)
    out = kernel(x, R, L)
    proj = x.astype(np.float64) @ R.astype(np.float64).T
    idx = np.argmax(proj, axis=1)
    exp = L[idx]
    bad = (out != exp).any(axis=1).sum()
    print("rows mismatching exact-gather expectation:", int(bad))


# revision 6
# speedup vs baseline: 1.0045x; 1.0045x over previous
"""Trainium2 Bass kernel for nn_LookupFFN (vq_codebook).

reference:  proj = x @ R.T ; idx = argmax(proj, 1) ; out = L[idx]
  x: [16384, 1024] f32, R: [1024, 1024] f32, L: [1024, 1024] f32

Strategy (data-parallel over 8 NeuronCores, 2048 rows of x per core):
  - The argmax needs full-fp32-class precision (real top-2 margins go down
    to 6.8e-4 while proj ~ +-130): a plain bf16 or fp32r (tf32-class)
    matmul flips rows.  fp32 matmul runs at 1/4 PE rate, so instead use a
    3-term bf16 split computed on host:
        x = xh + xl, R = Rh + Rl  (bf16 splits)
        x @ R.T ~= xh@Rh.T + xh@Rl.T + xl@Rh.T   (error ~1e-4, fp32-class)
    All three terms accumulate into the same PSUM tile at full bf16 PE
    rate (3 cycles/row total vs 4 for fp32).
  - Row-max + argmax via VectorE max/max_index straight from PSUM.
  - out rows fetched exactly (fp32) with a GPSIMD indirect DMA gather of
    L rows by the computed indices.
Perf notes:
  - R splits are loaded as 16 separate per-k-tile chunk DMAs (not 2 big
    4MB DMAs) so the first matmuls start ~1.5us in instead of ~18us.
  - Term/k loop is outer, bucket-half inner: each stationary lhsT tile is
    loaded once and streams both 512-wide halves.
"""
import sys

if "/opt/trn_rl_repo" not in sys.path:
    sys.path.insert(0, "/opt/trn_rl_repo")

import ml_dtypes
import numpy as np

import concourse.bass as bass
import concourse.tile as tile
from concourse import bacc, mybir
from concourse.bass import IndirectOffsetOnAxis
from concourse.bass_utils import run_bass_kernel_spmd

F32 = mybir.dt.float32
BF16 = mybir.dt.bfloat16
U32 = mybir.dt.uint32

N = 16384
D = 1024
NB = 1024  # buckets
DOUT = 1024
NCORES = 8
NSHARD = N // NCORES  # 2048 rows per core
KT = D // 128  # 8 k-tiles
NTILES = NSHARD // 128  # 16 n-tiles per core

_CACHED = {}


def build_nc(n_bufs: int = 5):
    nc = bacc.Bacc("TRN2", target_bir_lowering=False, debug=False)
    xh = nc.declare_dram_parameter("xh", [D, NSHARD], BF16, isOutput=False)
    xl = nc.declare_dram_parameter("xl", [D, NSHARD], BF16, isOutput=False)
    rh = nc.declare_dram_parameter("rh", [D, NB], BF16, isOutput=False)
    rl = nc.declare_dram_parameter("rl", [D, NB], BF16, isOutput=False)
    L = nc.declare_dram_parameter("L", [NB, DOUT], F32, isOutput=False)
    out = nc.declare_dram_parameter("out", [NSHARD, DOUT], F32, isOutput=True)

    with tile.TileContext(nc) as tc:
        with (
            tc.tile_pool(name="rpool", bufs=1) as rpool,
            tc.tile_pool(name="xpool", bufs=n_bufs) as xpool,
            tc.tile_pool(name="gpool", bufs=n_bufs) as gpool,
            tc.tile_pool(name="ipool", bufs=n_bufs) as ipool,
            tc.tile_pool(name="ps", bufs=4, space="PSUM") as ps,
        ):
            # R splits resident in SBUF, one tile per k-chunk so the first
            # matmuls only wait on their own chunk's DMA. Issued on the
            # scalar engine's HWDGE queue so they don't serialize ahead of
            # the x-tile loads on the sync queue (~0.7us issue cost each).
            rh_sb = []
            rl_sb = []
            for k2 in range(KT // 2):
                t_ = rpool.tile([128, 2, NB], BF16, tag=f"rh{k2}")
                nc.scalar.dma_start(
                    out=t_[:],
                    in_=rh[k2 * 256 : (k2 + 1) * 256, :].rearrange(
                        "(k p) b -> p k b", k=2
                    ),
                )
                rh_sb.extend([t_[:, 0, :], t_[:, 1, :]])
            for k2 in range(KT // 2):
                t_ = rpool.tile([128, 2, NB], BF16, tag=f"rl{k2}")
                nc.scalar.dma_start(
                    out=t_[:],
                    in_=rl[k2 * 256 : (k2 + 1) * 256, :].rearrange(
                        "(k p) b -> p k b", k=2
                    ),
                )
                rl_sb.extend([t_[:, 0, :], t_[:, 1, :]])

            # Software pipeline: per tile, run the two rh-only terms
            # (xh@Rh, xl@Rh) immediately, but defer the xh@Rl term (and
            # the tile's epilogue) by PIPE_DEPTH tiles.  At kernel start
            # this gives PE ~20us of rl-free work while the rl chunks are
            # still in flight behind rh on the DMA queues.
            PIPE_DEPTH = 3

            def finish_tile(t, proj, xh_sb):
                c0 = t * 128
                for i, k in enumerate(range(KT)):
                    for bh in range(2):
                        bs = bh * 512
                        nc.tensor.matmul(
                            proj[:, bs : bs + 512],
                            lhsT=xh_sb[:, k, :],
                            rhs=rl_sb[k][:, bs : bs + 512],
                            start=False,
                            stop=(i == KT - 1),
                        )
                max8 = ipool.tile([128, 8], F32, tag="max8")
                idx8 = ipool.tile([128, 8], U32, tag="idx8")
                nc.vector.max(max8[:], proj[:])
                nc.vector.max_index(idx8[:], max8[:], proj[:])

                g_sb = gpool.tile([128, DOUT], F32, tag="g")
                nc.gpsimd.indirect_dma_start(
                    out=g_sb[:],
                    out_offset=None,
                    in_=L[:],
                    in_offset=IndirectOffsetOnAxis(ap=idx8[:, 0:1], axis=0),
                )
                nc.sync.dma_start(out=out[c0 : c0 + 128, :], in_=g_sb[:])

            pend = []
            for t in range(NTILES):
                c0 = t * 128
                xh_sb = xpool.tile([128, KT, 128], BF16, tag="xh")
                xl_sb = xpool.tile([128, KT, 128], BF16, tag="xl")
                nc.sync.dma_start(
                    out=xh_sb[:],
                    in_=xh[:, c0 : c0 + 128].rearrange("(k p) j -> p k j", k=KT),
                )
                nc.sync.dma_start(
                    out=xl_sb[:],
                    in_=xl[:, c0 : c0 + 128].rearrange("(k p) j -> p k j", k=KT),
                )

                proj = ps.tile([128, NB], F32, tag="proj")
                steps = [(xh_sb, rh_sb, k) for k in range(KT)] + [
                    (xl_sb, rh_sb, k) for k in range(KT)
                ]
                for i, (xs, rs, k) in enumerate(steps):
                    for bh in range(2):
                        bs = bh * 512
                        nc.tensor.matmul(
                            proj[:, bs : bs + 512],
                            lhsT=xs[:, k, :],
                            rhs=rs[k][:, bs : bs + 512],
                            start=(i == 0),
                            stop=False,
                        )
                pend.append((t, proj, xh_sb))
                if len(pend) > PIPE_DEPTH - 1:
                    finish_tile(*pend.pop(0))
            while pend:
                finish_tile(*pend.pop(0))
    nc.compile()
    return nc


def _get_nc():
    if "nc" not in _CACHED:
        _CACHED["nc"] = build_nc()
    return _CACHED["nc"]


def _prep_inputs(x, R, L):
    """Host-side split + transpose. Returns per-core input maps."""
    x = np.ascontiguousarray(x, dtype=np.float32)
    R = np.ascontiguousarray(R, dtype=np.float32)
    L = np.ascontiguousarray(L, dtype=np.float32)

    xh = x.astype(ml_dtypes.bfloat16)
    xl = (x - xh.astype(np.float32)).astype(ml_dtypes.bfloat16)
    Rh = R.astype(ml_dtypes.bfloat16)
    Rl = (R - Rh.astype(np.float32)).astype(ml_dtypes.bfloat16)

    xhT = np.ascontiguousarray(xh.T)  # [D, N]
    xlT = np.ascontiguousarray(xl.T)
    rhT = np.ascontiguousarray(Rh.T)  # [D, NB]
    rlT = np.ascontiguousarray(Rl.T)

    in_maps = []
    for c in range(NCORES):
        s = slice(c * NSHARD, (c + 1) * NSHARD)
        in_maps.append(
            {
                "xh": np.ascontiguousarray(xhT[:, s]),
                "xl": np.ascontiguousarray(xlT[:, s]),
                "rh": rhT,
                "rl": rlT,
                "L": L,
            }
        )
    return in_maps


def run(x, R, L, trace=False, **kw):
    nc = _get_nc()
    in_maps = _prep_inputs(x, R, L)
    res = run_bass_kernel_spmd(
        nc, in_maps, core_ids=list(range(NCORES)), trace=trace, **kw
    )
    out = np.concatenate([res.results[c]["out"] for c in range(NCORES)], axis=0)
    return out, res


def kernel(x, R, L):
    out, _ = run(x, R, L, trace=False)
    return out


if __name__ == "__main__":
    rng = np.random.default_rng(0)
    x = rng.standard_normal((N, D), dtype=np.float32)
    R = rng.standard_normal((NB, D), dtype=np.float32)
    L = rng.standard_normal((NB, DOUT), dtype=np.float32)
    out = kernel(x, R, L)
    proj = x.astype(np.float64) @ R.astype(np.float64).T
    idx = np.argmax(proj, axis=1)
    exp = L[idx]
    bad = (out != exp).any(axis=1).sum()
    print("rows mismatching exact-gather expectation:", int(bad))
